# revision 48
# baseline (speedup 1.0000x reference)
import sys

if "/opt/trn_rl_repo" not in sys.path:
    sys.path.insert(0, "/opt/trn_rl_repo")

import numpy as np

B, S, V, D = 256, 512, 100, 64
NCORES = 8
R = B // NCORES  # rows per core

# const tile column layout (f32 [128, CW])
C_W1R0 = 0
C_W1R1 = 64
C_B1 = 128
C_W2 = 256
C_ID = 320
C_VIDX = 448
C_B2C = 449
CW = 450

_CACHE = {}
LAST_RESULT = None


def _emit(ctx, nc, tc, idsf, consts, out):
    from concourse import bass

    mybir = bass.mybir
    f32 = mybir.dt.float32
    f32r = mybir.dt.float32r
    bf16 = mybir.dt.bfloat16
    alu = mybir.AluOpType

    consts_p = ctx.enter_context(tc.tile_pool(name="cst", bufs=1))
    oh_p = ctx.enter_context(tc.tile_pool(name="oh", bufs=6))
    trash_p = ctx.enter_context(tc.tile_pool(name="trash", bufs=2))
    hist_p = ctx.enter_context(tc.tile_pool(name="hist", bufs=6))
    mlp_p = ctx.enter_context(tc.tile_pool(name="mlp", bufs=4))
    gout_p = ctx.enter_context(tc.tile_pool(name="gout", bufs=4))
    bc_p = ctx.enter_context(tc.tile_pool(name="bc", bufs=6))
    ps_ab = ctx.enter_context(tc.tile_pool(name="ps_ab", bufs=1, space="PSUM"))
    ps_mlp = ctx.enter_context(tc.tile_pool(name="ps_mlp", bufs=2, space="PSUM"))
    ps_g = ctx.enter_context(tc.tile_pool(name="ps_g", bufs=2, space="PSUM"))

    cst = consts_p.tile([128, CW], f32)
    # 32-partition chunks: a 128-partition DMA fans out across 4 HWDGE
    # queues and consumers would need 4 sync waits (HW allows 1).
    # Alternate queue engines so the four chunks transfer in parallel pairs.
    for k, p in enumerate(range(0, 128, 32)):
        qeng = nc.scalar if k % 2 == 0 else nc.sync
        qeng.dma_start(out=cst[p : p + 32, :], in_=consts[p : p + 32, :])


    w1r0 = cst[0:V, C_W1R0 : C_W1R0 + D]
    w1r1 = cst[0:V, C_W1R1 : C_W1R1 + D]
    b1b = cst[0:V, C_B1 : C_B1 + D]
    w2 = cst[0:D, C_W2 : C_W2 + D]
    ident = cst[0:V, C_ID : C_ID + V]
    vidx = cst[:, C_VIDX : C_VIDX + 1]
    b2c = cst[:, C_B2C : C_B2C + 1]

    # startup absorbers: every engine waits each input-DMA queue sem once
    # (HW allows 1 sync wait per instruction; wide DMAs fan out over queues)
    dvedum = trash_p.tile([1, 1], f32)
    actdum = trash_p.tile([1, 1], f32)
    pooldum = trash_p.tile([1, 1], f32)
    # base partition must be in {0,32,64}: chunk 3 is reached by a 64:128
    # span once chunk 2's queue wait is already absorbed
    for p0, p1 in ((0, 1), (32, 33), (64, 65), (64, 128)):
        c1 = cst[p0:p1, 0:1]
        cd = actdum if p1 - p0 == 1 else trash_p.tile([64, 1], f32)
        nc.scalar.copy(out=cd, in_=c1)
        pd = pooldum if p1 - p0 == 1 else trash_p.tile([64, 1], f32)
        nc.gpsimd.tensor_scalar_max(pd, c1, 0.0)
        dd = dvedum if p1 - p0 == 1 else trash_p.tile([64, 1], f32)
        nc.vector.tensor_tensor(out=dd, in0=c1, in1=c1, op=alu.add)

    w2b = consts_p.tile([D, D], bf16)
    nc.scalar.copy(out=w2b, in_=w2)
    w1r0b = consts_p.tile([V, D], bf16)
    nc.scalar.copy(out=w1r0b, in_=w1r0)
    w1r1b = consts_p.tile([V, D], bf16)
    nc.scalar.copy(out=w1r1b, in_=w1r1)
    b1bb = consts_p.tile([V, D], bf16)
    nc.scalar.copy(out=b1bb, in_=b1b)
    identb = consts_p.tile([V, V], bf16)
    nc.scalar.copy(out=identb, in_=ident)

    def bcast_dma(r):
        # replicate ids row r (src | dst) to V partitions via broadcast DMA
        bc = bc_p.tile([V, 2 * S], bf16, tag="bc")
        if r == 0:
            # startup absorbers: PE waits each cst-DMA queue sem once
            ps_a = ps_ab.tile([1, 1], f32)
            for p0, p1 in ((0, 1), (32, 33), (64, 65), (64, 128)):
                c1 = cst[p0:p1, 0:1]
                nc.tensor.matmul(
                    out=ps_a, lhsT=c1, rhs=c1, skip_group_check=True
                )
        nc.sync.dma_start(
            out=bc[:, 0:S], in_=idsf[r : r + 1, 0:S].broadcast_to((V, S))
        )
        nc.sync.dma_start(
            out=bc[:, S : 2 * S],
            in_=idsf[r : r + 1, S : 2 * S].broadcast_to((V, S)),
        )
        return bc

    from collections import deque

    pend = deque()
    pend.append(bcast_dma(0))
    pend.append(bcast_dma(1))
    ohs = {}
    hrelus = {}
    tabs = {}
    gout = None
    for i in range(R + 2):
        if i < R:
            # stage B/C (row i): one-hot + histogram + MLP up to relu
            bc = pend.popleft()
            oh = oh_p.tile([V, 2 * S], bf16)
            h_s = hist_p.tile([128, 1], f32)
            h_d = hist_p.tile([128, 1], f32)
            nc.vector.tensor_scalar(
                out=oh[:, 0:S], in0=bc[:, 0:S], scalar1=vidx[0:V, :],
                scalar2=None, op0=alu.is_equal, op1=alu.add,
                accum_out=h_s[0:V, :],
            )
            nc.vector.tensor_scalar(
                out=oh[:, S : 2 * S], in0=bc[:, S : 2 * S], scalar1=vidx[0:V, :],
                scalar2=None, op0=alu.is_equal, op1=alu.add,
                accum_out=h_d[0:V, :],
            )
            ohs[i] = oh
            if i + 2 < R:
                pend.append(bcast_dma(i + 2))

            # padding id 0 contributes zero features to the MLP
            nc.gpsimd.memset(h_s[0:1, :], 0.0)
            nc.gpsimd.memset(h_d[0:1, :], 0.0)

            tmp = mlp_p.tile([V, D], bf16)
            nc.vector.scalar_tensor_tensor(
                out=tmp, in0=w1r0b, scalar=h_s[0:V, :], in1=b1bb,
                op0=alu.mult, op1=alu.add,
            )
            hpre = mlp_p.tile([V, D], bf16)
            nc.vector.scalar_tensor_tensor(
                out=hpre, in0=w1r1b, scalar=h_d[0:V, :], in1=tmp,
                op0=alu.mult, op1=alu.add,
            )
            hrelus[i] = hpre

        if 1 <= i <= R:
            # stage D (row i-1): table[v,:] = relu(...) @ W2
            r = i - 1
            hpre_r = hrelus.pop(r)
            pst = ps_mlp.tile([D, V], bf16, tag="mlp")
            nc.tensor.transpose(pst, hpre_r, identb)
            hT = mlp_p.tile([D, V], bf16)
            nc.scalar.activation(
                out=hT, in_=pst, func=mybir.ActivationFunctionType.Relu
            )
            pstab = ps_mlp.tile([V, D], f32, tag="mlp")
            nc.tensor.matmul(out=pstab, lhsT=hT, rhs=w2b)
            tab = mlp_p.tile([V, D], bf16)
            nc.scalar.copy(out=tab, in_=pstab)
            tabs[r] = tab

        if i >= 2:
            # stage E/F (row i-2): gather + psum->sbuf (+b2) + DMA per 2 rows
            r = i - 2
            oh = ohs.pop(r)
            tab = tabs.pop(r)
            ps_gs = ps_g.tile([128, S], f32, bufs=3)
            ps_gd = ps_g.tile([128, S], f32)
            nc.tensor.matmul(out=ps_gs[0:D, :], lhsT=tab, rhs=oh[0:V, 0:S])
            nc.tensor.matmul(out=ps_gd[0:D, :], lhsT=tab, rhs=oh[0:V, S : 2 * S])

            g = r % 2
            if g == 0:
                gout = gout_p.tile([D, 4 * S], f32)
            nc.scalar.activation(
                out=gout[:, g * S : (g + 1) * S], in_=ps_gs[0:D, :],
                func=mybir.ActivationFunctionType.Identity, bias=b2c[0:D, :],
                scale=1.0,
            )
            nc.scalar.activation(
                out=gout[:, (2 + g) * S : (3 + g) * S], in_=ps_gd[0:D, :],
                func=mybir.ActivationFunctionType.Identity, bias=b2c[0:D, :],
                scale=1.0,
            )
            if g == 1:
                win = slice((r - 1) * S, (r + 1) * S)
                nc.gpsimd.dma_start(out=out[0:D, win], in_=gout[:, 0 : 2 * S])
                nc.gpsimd.dma_start(
                    out=out[D : 2 * D, win], in_=gout[:, 2 * S : 4 * S]
                )


def _build_module():
    from contextlib import ExitStack

    from concourse import bacc, bass, tile

    mybir = bass.mybir
    # Bacc.finalize() runs generate_event_semaphores, splitting sync waits
    # to the HW limit of 1 per instruction (raw Bass skips that pass)
    nc = bacc.Bacc(
        "TRN2", target_bir_lowering=False, debug=False, num_devices=NCORES
    )
    idsf = nc.dram_tensor(
        "idsf", [R, 2 * S], mybir.dt.bfloat16, kind="ExternalInput"
    ).ap()
    consts = nc.dram_tensor(
        "consts", [128, CW], mybir.dt.float32, kind="ExternalInput"
    ).ap()
    out = nc.dram_tensor(
        "out", [128, R * S], mybir.dt.float32, kind="ExternalOutput"
    ).ap()

    with tile.TileContext(nc) as tc:
        with ExitStack() as ctx:
            _emit(ctx, nc, tc, idsf, consts, out)
    nc.finalize()
    return nc


def get_module():
    if "nc" not in _CACHE:
        _CACHE["nc"] = _build_module()
    return _CACHE["nc"]


def _build_consts(W1, b1, W2, b2):
    c = np.zeros((128, CW), np.float32)
    c[:, C_W1R0 : C_W1R0 + D] = W1[0]
    c[:, C_W1R1 : C_W1R1 + D] = W1[1]
    c[:, C_B1 : C_B1 + D] = b1
    c[0:D, C_W2 : C_W2 + D] = W2
    c[:, C_ID : C_ID + 128] = np.eye(128, dtype=np.float32)
    c[:, C_VIDX] = np.arange(128, dtype=np.float32)
    c[0:D, C_B2C] = b2
    c[D : 2 * D, C_B2C] = b2
    return c


def kernel(**inputs):
    global LAST_RESULT
    import ml_dtypes

    from concourse import bass_utils

    src = np.asarray(inputs["src_neighbor_ids"])
    dst = np.asarray(inputs["dst_neighbor_ids"])
    W1 = np.asarray(inputs["W1"], np.float32)
    b1 = np.asarray(inputs["b1"], np.float32)
    W2 = np.asarray(inputs["W2"], np.float32)
    b2 = np.asarray(inputs["b2"], np.float32)

    consts = _build_consts(W1, b1, W2, b2)
    idsf = np.concatenate(
        [src.astype(np.float32), dst.astype(np.float32)], axis=1
    ).astype(ml_dtypes.bfloat16)

    in_maps = []
    for c in range(NCORES):
        sl = slice(c * R, (c + 1) * R)
        in_maps.append({"idsf": idsf[sl], "consts": consts})

    nc = get_module()
    import os

    trace = bool(int(os.environ.get("KERNEL_TRACE", "0")))
    res = bass_utils.run_bass_kernel_spmd(
        nc, in_maps, core_ids=list(range(NCORES)), trace=trace
    )
    LAST_RESULT = res

    src_feat = np.empty((B, S, D), np.float32)
    dst_feat = np.empty((B, S, D), np.float32)
    for c in range(NCORES):
        o = res.results[c]["out"].reshape(128, R, S)
        sl = slice(c * R, (c + 1) * R)
        src_feat[sl] = o[0:D].transpose(1, 2, 0)
        dst_feat[sl] = o[D : 2 * D].transpose(1, 2, 0)
    return src_feat, dst_feat



# revision 49
# speedup vs baseline: 1.0094x; 1.0094x over previous
import sys

if "/opt/trn_rl_repo" not in sys.path:
    sys.path.insert(0, "/opt/trn_rl_repo")

import numpy as np

B, S, V, D = 256, 512, 100, 64
NCORES = 8
R = B // NCORES  # rows per core

# const tile column layout (f32 [128, CW])
C_W1R0 = 0
C_W1R1 = 64
C_B1 = 128
C_W2 = 256
C_ID = 320
C_VIDX = 448
C_B2C = 449
CW = 450

_CACHE = {}
LAST_RESULT = None


def _emit(ctx, nc, tc, idsf, consts, out):
    from concourse import bass

    mybir = bass.mybir
    f32 = mybir.dt.float32
    f32r = mybir.dt.float32r
    bf16 = mybir.dt.bfloat16
    alu = mybir.AluOpType

    consts_p = ctx.enter_context(tc.tile_pool(name="cst", bufs=1))
    oh_p = ctx.enter_context(tc.tile_pool(name="oh", bufs=6))
    trash_p = ctx.enter_context(tc.tile_pool(name="trash", bufs=2))
    hist_p = ctx.enter_context(tc.tile_pool(name="hist", bufs=6))
    mlp_p = ctx.enter_context(tc.tile_pool(name="mlp", bufs=4))
    gout_p = ctx.enter_context(tc.tile_pool(name="gout", bufs=4))
    bc_p = ctx.enter_context(tc.tile_pool(name="bc", bufs=6))
    ps_ab = ctx.enter_context(tc.tile_pool(name="ps_ab", bufs=1, space="PSUM"))
    ps_mlp = ctx.enter_context(tc.tile_pool(name="ps_mlp", bufs=2, space="PSUM"))
    ps_g = ctx.enter_context(tc.tile_pool(name="ps_g", bufs=2, space="PSUM"))

    cst = consts_p.tile([128, CW], f32)
    # 32-partition chunks: a 128-partition DMA fans out across 4 HWDGE
    # queues and consumers would need 4 sync waits (HW allows 1)
    for p in range(0, 128, 32):
        nc.scalar.dma_start(out=cst[p : p + 32, :], in_=consts[p : p + 32, :])


    w1r0 = cst[0:V, C_W1R0 : C_W1R0 + D]
    w1r1 = cst[0:V, C_W1R1 : C_W1R1 + D]
    b1b = cst[0:V, C_B1 : C_B1 + D]
    w2 = cst[0:D, C_W2 : C_W2 + D]
    ident = cst[0:V, C_ID : C_ID + V]
    vidx = cst[:, C_VIDX : C_VIDX + 1]
    b2c = cst[:, C_B2C : C_B2C + 1]

    # startup absorbers: every engine waits each input-DMA queue sem once
    # (HW allows 1 sync wait per instruction; wide DMAs fan out over queues)
    dvedum = trash_p.tile([1, 1], f32)
    actdum = trash_p.tile([1, 1], f32)
    pooldum = trash_p.tile([1, 1], f32)
    # base partition must be in {0,32,64}: chunk 3 is reached by a 64:128
    # span once chunk 2's queue wait is already absorbed
    for p0, p1 in ((0, 1), (32, 33), (64, 65), (64, 128)):
        c1 = cst[p0:p1, 0:1]
        cd = actdum if p1 - p0 == 1 else trash_p.tile([64, 1], f32)
        nc.scalar.copy(out=cd, in_=c1)
        pd = pooldum if p1 - p0 == 1 else trash_p.tile([64, 1], f32)
        nc.gpsimd.tensor_scalar_max(pd, c1, 0.0)
        dd = dvedum if p1 - p0 == 1 else trash_p.tile([64, 1], f32)
        nc.vector.tensor_tensor(out=dd, in0=c1, in1=c1, op=alu.add)

    w2b = consts_p.tile([D, D], bf16)
    nc.scalar.copy(out=w2b, in_=w2)
    w1r0b = consts_p.tile([V, D], bf16)
    nc.scalar.copy(out=w1r0b, in_=w1r0)
    w1r1b = consts_p.tile([V, D], bf16)
    nc.scalar.copy(out=w1r1b, in_=w1r1)
    b1bb = consts_p.tile([V, D], bf16)
    nc.scalar.copy(out=b1bb, in_=b1b)
    identb = consts_p.tile([V, V], bf16)
    nc.scalar.copy(out=identb, in_=ident)

    def bcast_dma(r):
        # replicate ids row r (src | dst) to V partitions via broadcast DMA
        bc = bc_p.tile([V, 2 * S], bf16, tag="bc")
        if r == 0:
            # startup absorbers: PE waits each cst-DMA queue sem once
            ps_a = ps_ab.tile([1, 1], f32)
            for p0, p1 in ((0, 1), (32, 33), (64, 65), (64, 128)):
                c1 = cst[p0:p1, 0:1]
                nc.tensor.matmul(
                    out=ps_a, lhsT=c1, rhs=c1, skip_group_check=True
                )
        nc.sync.dma_start(
            out=bc[:, 0:S], in_=idsf[r : r + 1, 0:S].broadcast_to((V, S))
        )
        nc.sync.dma_start(
            out=bc[:, S : 2 * S],
            in_=idsf[r : r + 1, S : 2 * S].broadcast_to((V, S)),
        )
        return bc

    from collections import deque

    pend = deque()
    pend.append(bcast_dma(0))
    pend.append(bcast_dma(1))
    ohs = {}
    hrelus = {}
    tabs = {}
    gout = None
    for i in range(R + 2):
        if i < R:
            # stage B/C (row i): one-hot + histogram + MLP up to relu
            bc = pend.popleft()
            oh = oh_p.tile([V, 2 * S], bf16)
            h_s = hist_p.tile([128, 1], f32)
            h_d = hist_p.tile([128, 1], f32)
            nc.vector.tensor_scalar(
                out=oh[:, 0:S], in0=bc[:, 0:S], scalar1=vidx[0:V, :],
                scalar2=None, op0=alu.is_equal, op1=alu.add,
                accum_out=h_s[0:V, :],
            )
            nc.vector.tensor_scalar(
                out=oh[:, S : 2 * S], in0=bc[:, S : 2 * S], scalar1=vidx[0:V, :],
                scalar2=None, op0=alu.is_equal, op1=alu.add,
                accum_out=h_d[0:V, :],
            )
            ohs[i] = oh
            if i + 2 < R:
                pend.append(bcast_dma(i + 2))

            # padding id 0 contributes zero features to the MLP
            nc.gpsimd.memset(h_s[0:1, :], 0.0)
            nc.gpsimd.memset(h_d[0:1, :], 0.0)

            tmp = mlp_p.tile([V, D], bf16)
            nc.vector.scalar_tensor_tensor(
                out=tmp, in0=w1r0b, scalar=h_s[0:V, :], in1=b1bb,
                op0=alu.mult, op1=alu.add,
            )
            hpre = mlp_p.tile([V, D], bf16)
            nc.vector.scalar_tensor_tensor(
                out=hpre, in0=w1r1b, scalar=h_d[0:V, :], in1=tmp,
                op0=alu.mult, op1=alu.add,
            )
            hrelus[i] = hpre

        if 1 <= i <= R:
            # stage D (row i-1): table[v,:] = relu(...) @ W2
            r = i - 1
            hpre_r = hrelus.pop(r)
            pst = ps_mlp.tile([D, V], bf16, tag="mlp")
            nc.tensor.transpose(pst, hpre_r, identb)
            hT = mlp_p.tile([D, V], bf16)
            nc.scalar.activation(
                out=hT, in_=pst, func=mybir.ActivationFunctionType.Relu
            )
            pstab = ps_mlp.tile([V, D], f32, tag="mlp")
            nc.tensor.matmul(out=pstab, lhsT=hT, rhs=w2b)
            tab = mlp_p.tile([V, D], bf16)
            nc.scalar.copy(out=tab, in_=pstab)
            tabs[r] = tab

        if i >= 2:
            # stage E/F (row i-2): gather + psum->sbuf (+b2) + DMA per 2 rows
            r = i - 2
            oh = ohs.pop(r)
            tab = tabs.pop(r)
            ps_gs = ps_g.tile([128, S], f32, bufs=3)
            ps_gd = ps_g.tile([128, S], f32)
            nc.tensor.matmul(out=ps_gs[0:D, :], lhsT=tab, rhs=oh[0:V, 0:S])
            nc.tensor.matmul(out=ps_gd[0:D, :], lhsT=tab, rhs=oh[0:V, S : 2 * S])

            g = r % 2
            if g == 0:
                gout = gout_p.tile([D, 4 * S], f32)
            nc.scalar.activation(
                out=gout[:, g * S : (g + 1) * S], in_=ps_gs[0:D, :],
                func=mybir.ActivationFunctionType.Identity, bias=b2c[0:D, :],
                scale=1.0,
            )
            nc.scalar.activation(
                out=gout[:, (2 + g) * S : (3 + g) * S], in_=ps_gd[0:D, :],
                func=mybir.ActivationFunctionType.Identity, bias=b2c[0:D, :],
                scale=1.0,
            )
            if g == 1:
                win = slice((r - 1) * S, (r + 1) * S)
                nc.gpsimd.dma_start(out=out[0:D, win], in_=gout[:, 0 : 2 * S])
                nc.gpsimd.dma_start(
                    out=out[D : 2 * D, win], in_=gout[:, 2 * S : 4 * S]
                )


def _build_module():
    from contextlib import ExitStack

    from concourse import bacc, bass, tile

    mybir = bass.mybir
    # Bacc.finalize() runs generate_event_semaphores, splitting sync waits
    # to the HW limit of 1 per instruction (raw Bass skips that pass)
    nc = bacc.Bacc(
        "TRN2", target_bir_lowering=False, debug=False, num_devices=NCORES
    )
    idsf = nc.dram_tensor(
        "idsf", [R, 2 * S], mybir.dt.bfloat16, kind="ExternalInput"
    ).ap()
    consts = nc.dram_tensor(
        "consts", [128, CW], mybir.dt.float32, kind="ExternalInput"
    ).ap()
    out = nc.dram_tensor(
        "out", [128, R * S], mybir.dt.float32, kind="ExternalOutput"
    ).ap()

    with tile.TileContext(nc) as tc:
        with ExitStack() as ctx:
            _emit(ctx, nc, tc, idsf, consts, out)
    nc.finalize()
    return nc


def get_module():
    if "nc" not in _CACHE:
        _CACHE["nc"] = _build_module()
    return _CACHE["nc"]


def _build_consts(W1, b1, W2, b2):
    c = np.zeros((128, CW), np.float32)
    c[:, C_W1R0 : C_W1R0 + D] = W1[0]
    c[:, C_W1R1 : C_W1R1 + D] = W1[1]
    c[:, C_B1 : C_B1 + D] = b1
    c[0:D, C_W2 : C_W2 + D] = W2
    c[:, C_ID : C_ID + 128] = np.eye(128, dtype=np.float32)
    c[:, C_VIDX] = np.arange(128, dtype=np.float32)
    c[0:D, C_B2C] = b2
    c[D : 2 * D, C_B2C] = b2
    return c


def kernel(**inputs):
    global LAST_RESULT
    import ml_dtypes

    from concourse import bass_utils

    src = np.asarray(inputs["src_neighbor_ids"])
    dst = np.asarray(inputs["dst_neighbor_ids"])
    W1 = np.asarray(inputs["W1"], np.float32)
    b1 = np.asarray(inputs["b1"], np.float32)
    W2 = np.asarray(inputs["W2"], np.float32)
    b2 = np.asarray(inputs["b2"], np.float32)

    consts = _build_consts(W1, b1, W2, b2)
    idsf = np.concatenate(
        [src.astype(np.float32), dst.astype(np.float32)], axis=1
    ).astype(ml_dtypes.bfloat16)

    in_maps = []
    for c in range(NCORES):
        sl = slice(c * R, (c + 1) * R)
        in_maps.append({"idsf": idsf[sl], "consts": consts})

    nc = get_module()
    import os

    trace = bool(int(os.environ.get("KERNEL_TRACE", "0")))
    res = bass_utils.run_bass_kernel_spmd(
        nc, in_maps, core_ids=list(range(NCORES)), trace=trace
    )
    LAST_RESULT = res

    src_feat = np.empty((B, S, D), np.float32)
    dst_feat = np.empty((B, S, D), np.float32)
    for c in range(NCORES):
        o = res.results[c]["out"].reshape(128, R, S)
        sl = slice(c * R, (c + 1) * R)
        src_feat[sl] = o[0:D].transpose(1, 2, 0)
        dst_feat[sl] = o[D : 2 * D].transpose(1, 2, 0)
    return src_feat, dst_feat



# revision 52
# speedup vs baseline: 1.0119x; 1.0025x over previous
import sys

if "/opt/trn_rl_repo" not in sys.path:
    sys.path.insert(0, "/opt/trn_rl_repo")

import numpy as np

B, S, V, D = 256, 512, 100, 64
NCORES = 8
R = B // NCORES  # rows per core

# const tile column layout (bf16 [128, CW])
C_W1R0 = 0
C_W1R1 = 64
C_B1 = 128
C_W2 = 192
C_ID = 256
CW = 384

_CACHE = {}
LAST_RESULT = None


def _emit(ctx, nc, tc, idsf, consts, cstf, out):
    from concourse import bass

    mybir = bass.mybir
    f32 = mybir.dt.float32
    f32r = mybir.dt.float32r
    bf16 = mybir.dt.bfloat16
    alu = mybir.AluOpType

    consts_p = ctx.enter_context(tc.tile_pool(name="cst", bufs=1))
    oh_p = ctx.enter_context(tc.tile_pool(name="oh", bufs=6))
    trash_p = ctx.enter_context(tc.tile_pool(name="trash", bufs=2))
    hist_p = ctx.enter_context(tc.tile_pool(name="hist", bufs=6))
    mlp_p = ctx.enter_context(tc.tile_pool(name="mlp", bufs=4))
    gout_p = ctx.enter_context(tc.tile_pool(name="gout", bufs=4))
    bc_p = ctx.enter_context(tc.tile_pool(name="bc", bufs=6))
    ps_ab = ctx.enter_context(tc.tile_pool(name="ps_ab", bufs=1, space="PSUM"))
    ps_mlp = ctx.enter_context(tc.tile_pool(name="ps_mlp", bufs=2, space="PSUM"))
    ps_g = ctx.enter_context(tc.tile_pool(name="ps_g", bufs=2, space="PSUM"))

    cst = consts_p.tile([128, CW], bf16)
    # 32-partition chunks: a 128-partition DMA fans out across 4 HWDGE
    # queues and consumers would need 4 sync waits (HW allows 1)
    for p in range(0, 128, 32):
        nc.scalar.dma_start(out=cst[p : p + 32, :], in_=consts[p : p + 32, :])

    w1r0b = cst[0:V, C_W1R0 : C_W1R0 + D]
    w1r1b = cst[0:V, C_W1R1 : C_W1R1 + D]
    b1bb = cst[0:V, C_B1 : C_B1 + D]
    w2b = cst[0:D, C_W2 : C_W2 + D]
    identb = cst[0:V, C_ID : C_ID + V]
    cf = consts_p.tile([128, 2], f32)
    nc.sync.dma_start(out=cf, in_=cstf)
    vidx = cf[:, 0:1]
    b2c = cf[:, 1:2]

    # startup absorbers: every engine waits each input-DMA queue sem once
    # (HW allows 1 sync wait per instruction; wide DMAs fan out over queues)
    dvedum = trash_p.tile([1, 1], bf16)
    actdum = trash_p.tile([1, 1], bf16)
    pooldum = trash_p.tile([1, 1], bf16)
    # base partition must be in {0,32,64}: chunk 3 is reached by a 64:128
    # span once chunk 2's queue wait is already absorbed
    for p0, p1 in ((0, 1), (32, 33), (64, 65), (64, 128)):
        c1 = cst[p0:p1, 0:1]
        cd = actdum if p1 - p0 == 1 else trash_p.tile([64, 1], bf16)
        nc.scalar.copy(out=cd, in_=c1)
        pd = pooldum if p1 - p0 == 1 else trash_p.tile([64, 1], bf16)
        nc.gpsimd.tensor_scalar_max(pd, c1, 0.0)
        dd = dvedum if p1 - p0 == 1 else trash_p.tile([64, 1], bf16)
        nc.vector.tensor_tensor(out=dd, in0=c1, in1=c1, op=alu.add)

    def bcast_dma(r):
        # replicate ids row r (src | dst) to V partitions via broadcast DMA
        bc = bc_p.tile([V, 2 * S], bf16, tag="bc")
        if r == 0:
            # startup absorbers: PE waits each cst-DMA queue sem once
            ps_a = ps_ab.tile([1, 1], f32)
            for p0, p1 in ((0, 1), (32, 33), (64, 65), (64, 128)):
                c1 = cst[p0:p1, 0:1]
                nc.tensor.matmul(
                    out=ps_a, lhsT=c1, rhs=c1, skip_group_check=True
                )
        nc.sync.dma_start(
            out=bc[:, 0:S], in_=idsf[r : r + 1, 0:S].broadcast_to((V, S))
        )
        nc.sync.dma_start(
            out=bc[:, S : 2 * S],
            in_=idsf[r : r + 1, S : 2 * S].broadcast_to((V, S)),
        )
        return bc

    from collections import deque

    pend = deque()
    pend.append(bcast_dma(0))
    pend.append(bcast_dma(1))
    ohs = {}
    hrelus = {}
    tabs = {}
    gout = None
    for i in range(R + 2):
        if i < R:
            # stage B/C (row i): one-hot + histogram + MLP up to relu
            bc = pend.popleft()
            oh = oh_p.tile([V, 2 * S], bf16)
            h_s = hist_p.tile([128, 1], f32)
            h_d = hist_p.tile([128, 1], f32)
            nc.vector.tensor_scalar(
                out=oh[:, 0:S], in0=bc[:, 0:S], scalar1=vidx[0:V, :],
                scalar2=None, op0=alu.is_equal, op1=alu.add,
                accum_out=h_s[0:V, :],
            )
            nc.vector.tensor_scalar(
                out=oh[:, S : 2 * S], in0=bc[:, S : 2 * S], scalar1=vidx[0:V, :],
                scalar2=None, op0=alu.is_equal, op1=alu.add,
                accum_out=h_d[0:V, :],
            )
            ohs[i] = oh
            if i + 2 < R:
                pend.append(bcast_dma(i + 2))

            # padding id 0 contributes zero features to the MLP
            nc.gpsimd.memset(h_s[0:1, :], 0.0)
            nc.gpsimd.memset(h_d[0:1, :], 0.0)

            tmp = mlp_p.tile([V, D], bf16)
            nc.vector.scalar_tensor_tensor(
                out=tmp, in0=w1r0b, scalar=h_s[0:V, :], in1=b1bb,
                op0=alu.mult, op1=alu.add,
            )
            hpre = mlp_p.tile([V, D], bf16)
            nc.vector.scalar_tensor_tensor(
                out=hpre, in0=w1r1b, scalar=h_d[0:V, :], in1=tmp,
                op0=alu.mult, op1=alu.add,
            )
            hrelus[i] = hpre

        if 1 <= i <= R:
            # stage D (row i-1): table[v,:] = relu(...) @ W2
            r = i - 1
            hpre_r = hrelus.pop(r)
            pst = ps_mlp.tile([D, V], bf16, tag="mlp")
            nc.tensor.transpose(pst, hpre_r, identb)
            hT = mlp_p.tile([D, V], bf16)
            nc.scalar.activation(
                out=hT, in_=pst, func=mybir.ActivationFunctionType.Relu
            )
            pstab = ps_mlp.tile([V, D], f32, tag="mlp")
            nc.tensor.matmul(out=pstab, lhsT=hT, rhs=w2b)
            tab = mlp_p.tile([V, D], bf16)
            nc.scalar.copy(out=tab, in_=pstab)
            tabs[r] = tab

        if i >= 2:
            # stage E/F (row i-2): gather + psum->sbuf (+b2) + DMA per 2 rows
            r = i - 2
            oh = ohs.pop(r)
            tab = tabs.pop(r)
            ps_gs = ps_g.tile([128, S], f32, bufs=3)
            ps_gd = ps_g.tile([128, S], f32)
            nc.tensor.matmul(out=ps_gs[0:D, :], lhsT=tab, rhs=oh[0:V, 0:S])
            nc.tensor.matmul(out=ps_gd[0:D, :], lhsT=tab, rhs=oh[0:V, S : 2 * S])

            g = r % 2
            if g == 0:
                gout = gout_p.tile([D, 4 * S], f32)
            nc.scalar.activation(
                out=gout[:, g * S : (g + 1) * S], in_=ps_gs[0:D, :],
                func=mybir.ActivationFunctionType.Identity, bias=b2c[0:D, :],
                scale=1.0,
            )
            nc.scalar.activation(
                out=gout[:, (2 + g) * S : (3 + g) * S], in_=ps_gd[0:D, :],
                func=mybir.ActivationFunctionType.Identity, bias=b2c[0:D, :],
                scale=1.0,
            )
            if g == 1:
                win = slice((r - 1) * S, (r + 1) * S)
                nc.gpsimd.dma_start(out=out[0:D, win], in_=gout[:, 0 : 2 * S])
                nc.gpsimd.dma_start(
                    out=out[D : 2 * D, win], in_=gout[:, 2 * S : 4 * S]
                )


def _build_module():
    from contextlib import ExitStack

    from concourse import bacc, bass, tile

    mybir = bass.mybir
    # Bacc.finalize() runs generate_event_semaphores, splitting sync waits
    # to the HW limit of 1 per instruction (raw Bass skips that pass)
    nc = bacc.Bacc(
        "TRN2", target_bir_lowering=False, debug=False, num_devices=NCORES
    )
    idsf = nc.dram_tensor(
        "idsf", [R, 2 * S], mybir.dt.bfloat16, kind="ExternalInput"
    ).ap()
    consts = nc.dram_tensor(
        "consts", [128, CW], mybir.dt.bfloat16, kind="ExternalInput"
    ).ap()
    cstf = nc.dram_tensor(
        "cstf", [128, 2], mybir.dt.float32, kind="ExternalInput"
    ).ap()
    out = nc.dram_tensor(
        "out", [128, R * S], mybir.dt.float32, kind="ExternalOutput"
    ).ap()

    with tile.TileContext(nc) as tc:
        with ExitStack() as ctx:
            _emit(ctx, nc, tc, idsf, consts, cstf, out)
    nc.finalize()
    return nc


def get_module():
    if "nc" not in _CACHE:
        _CACHE["nc"] = _build_module()
    return _CACHE["nc"]


def _build_consts(W1, b1, W2, b2):
    import ml_dtypes

    c = np.zeros((128, CW), np.float32)
    c[:, C_W1R0 : C_W1R0 + D] = W1[0]
    c[:, C_W1R1 : C_W1R1 + D] = W1[1]
    c[:, C_B1 : C_B1 + D] = b1
    c[0:D, C_W2 : C_W2 + D] = W2
    c[:, C_ID : C_ID + 128] = np.eye(128, dtype=np.float32)
    cf = np.zeros((128, 2), np.float32)
    cf[:, 0] = np.arange(128, dtype=np.float32)
    cf[0:D, 1] = b2
    cf[D : 2 * D, 1] = b2
    return c.astype(ml_dtypes.bfloat16), cf


def kernel(**inputs):
    global LAST_RESULT
    import ml_dtypes

    from concourse import bass_utils

    src = np.asarray(inputs["src_neighbor_ids"])
    dst = np.asarray(inputs["dst_neighbor_ids"])
    W1 = np.asarray(inputs["W1"], np.float32)
    b1 = np.asarray(inputs["b1"], np.float32)
    W2 = np.asarray(inputs["W2"], np.float32)
    b2 = np.asarray(inputs["b2"], np.float32)

    consts, cstf = _build_consts(W1, b1, W2, b2)
    idsf = np.concatenate(
        [src.astype(np.float32), dst.astype(np.float32)], axis=1
    ).astype(ml_dtypes.bfloat16)

    in_maps = []
    for c in range(NCORES):
        sl = slice(c * R, (c + 1) * R)
        in_maps.append(
            {"idsf": idsf[sl], "consts": consts, "cstf": cstf}
        )

    nc = get_module()
    import os

    trace = bool(int(os.environ.get("KERNEL_TRACE", "0")))
    res = bass_utils.run_bass_kernel_spmd(
        nc, in_maps, core_ids=list(range(NCORES)), trace=trace
    )
    LAST_RESULT = res

    src_feat = np.empty((B, S, D), np.float32)
    dst_feat = np.empty((B, S, D), np.float32)
    for c in range(NCORES):
        o = res.results[c]["out"].reshape(128, R, S)
        sl = slice(c * R, (c + 1) * R)
        src_feat[sl] = o[0:D].transpose(1, 2, 0)
        dst_feat[sl] = o[D : 2 * D].transpose(1, 2, 0)
    return src_feat, dst_feat



# revision 54
# speedup vs baseline: 1.0377x; 1.0255x over previous
import sys

if "/opt/trn_rl_repo" not in sys.path:
    sys.path.insert(0, "/opt/trn_rl_repo")

import numpy as np

B, S, V, D = 256, 512, 100, 64
NCORES = 8
R = B // NCORES  # rows per core

# const tile column layout (f32 [128, CW])
C_W1R0 = 0
C_W1R1 = 64
C_B1 = 128
C_W2 = 256
C_ID = 320
C_VIDX = 448
C_B2C = 449
CW = 450

_CACHE = {}
LAST_RESULT = None


def _emit(ctx, nc, tc, idsf, consts, out):
    from concourse import bass

    mybir = bass.mybir
    f32 = mybir.dt.float32
    f32r = mybir.dt.float32r
    bf16 = mybir.dt.bfloat16
    alu = mybir.AluOpType

    consts_p = ctx.enter_context(tc.tile_pool(name="cst", bufs=1))
    oh_p = ctx.enter_context(tc.tile_pool(name="oh", bufs=6))
    trash_p = ctx.enter_context(tc.tile_pool(name="trash", bufs=2))
    hist_p = ctx.enter_context(tc.tile_pool(name="hist", bufs=6))
    mlp_p = ctx.enter_context(tc.tile_pool(name="mlp", bufs=4))
    gout_p = ctx.enter_context(tc.tile_pool(name="gout", bufs=4))
    bc_p = ctx.enter_context(tc.tile_pool(name="bc", bufs=6))
    ps_ab = ctx.enter_context(tc.tile_pool(name="ps_ab", bufs=1, space="PSUM"))
    ps_mlp = ctx.enter_context(tc.tile_pool(name="ps_mlp", bufs=2, space="PSUM"))
    ps_g = ctx.enter_context(tc.tile_pool(name="ps_g", bufs=2, space="PSUM"))

    cst = consts_p.tile([128, CW], f32)
    # 32-partition chunks: a 128-partition DMA fans out across 4 HWDGE
    # queues and consumers would need 4 sync waits (HW allows 1)
    for p in range(0, 128, 32):
        nc.scalar.dma_start(out=cst[p : p + 32, :], in_=consts[p : p + 32, :])


    w1r0 = cst[0:V, C_W1R0 : C_W1R0 + D]
    w1r1 = cst[0:V, C_W1R1 : C_W1R1 + D]
    b1b = cst[0:V, C_B1 : C_B1 + D]
    w2 = cst[0:D, C_W2 : C_W2 + D]
    ident = cst[0:V, C_ID : C_ID + V]
    vidx = cst[:, C_VIDX : C_VIDX + 1]
    b2c = cst[:, C_B2C : C_B2C + 1]

    # startup absorbers: every engine waits each input-DMA queue sem once
    # (HW allows 1 sync wait per instruction; wide DMAs fan out over queues)
    dvedum = trash_p.tile([1, 1], f32)
    actdum = trash_p.tile([1, 1], f32)
    pooldum = trash_p.tile([1, 1], f32)
    # base partition must be in {0,32,64}: chunk 3 is reached by a 64:128
    # span once chunk 2's queue wait is already absorbed
    for p0, p1 in ((0, 1), (32, 33), (64, 65), (64, 128)):
        c1 = cst[p0:p1, 0:1]
        cd = actdum if p1 - p0 == 1 else trash_p.tile([64, 1], f32)
        nc.scalar.copy(out=cd, in_=c1)
        pd = pooldum if p1 - p0 == 1 else trash_p.tile([64, 1], f32)
        nc.gpsimd.tensor_scalar_max(pd, c1, 0.0)
        dd = dvedum if p1 - p0 == 1 else trash_p.tile([64, 1], f32)
        nc.vector.tensor_tensor(out=dd, in0=c1, in1=c1, op=alu.add)

    w2b = consts_p.tile([D, D], bf16)
    nc.scalar.copy(out=w2b, in_=w2)
    w1r0b = consts_p.tile([V, D], bf16)
    nc.scalar.copy(out=w1r0b, in_=w1r0)
    w1r1b = consts_p.tile([V, D], bf16)
    nc.scalar.copy(out=w1r1b, in_=w1r1)
    b1bb = consts_p.tile([V, D], bf16)
    nc.scalar.copy(out=b1bb, in_=b1b)
    identb = consts_p.tile([V, V], bf16)
    nc.scalar.copy(out=identb, in_=ident)

    def bcast_dma(r):
        # replicate ids row r (src | dst) to V partitions via broadcast DMA
        bc = bc_p.tile([V, 2 * S], bf16, tag="bc")
        if r == 0:
            # startup absorbers: PE waits each cst-DMA queue sem once
            ps_a = ps_ab.tile([1, 1], f32)
            for p0, p1 in ((0, 1), (32, 33), (64, 65), (64, 128)):
                c1 = cst[p0:p1, 0:1]
                nc.tensor.matmul(
                    out=ps_a, lhsT=c1, rhs=c1, skip_group_check=True
                )
        nc.sync.dma_start(
            out=bc[:, 0:S], in_=idsf[r : r + 1, 0:S].broadcast_to((V, S))
        )
        nc.sync.dma_start(
            out=bc[:, S : 2 * S],
            in_=idsf[r : r + 1, S : 2 * S].broadcast_to((V, S)),
        )
        return bc

    from collections import deque

    pend = deque()
    pend.append(bcast_dma(0))
    pend.append(bcast_dma(1))
    ohs = {}
    hrelus = {}
    tabs = {}
    gout = None
    for i in range(R + 2):
        if i < R:
            # stage B/C (row i): one-hot + histogram + MLP up to relu
            bc = pend.popleft()
            oh = oh_p.tile([V, 2 * S], bf16)
            h_s = hist_p.tile([128, 1], f32)
            h_d = hist_p.tile([128, 1], f32)
            nc.vector.tensor_scalar(
                out=oh[:, 0:S], in0=bc[:, 0:S], scalar1=vidx[0:V, :],
                scalar2=None, op0=alu.is_equal, op1=alu.add,
                accum_out=h_s[0:V, :],
            )
            nc.vector.tensor_scalar(
                out=oh[:, S : 2 * S], in0=bc[:, S : 2 * S], scalar1=vidx[0:V, :],
                scalar2=None, op0=alu.is_equal, op1=alu.add,
                accum_out=h_d[0:V, :],
            )
            ohs[i] = oh
            if i + 2 < R:
                pend.append(bcast_dma(i + 2))

            # padding id 0: W1 row 0 is zeroed host-side, so table row 0
            # evaluates to MLP(0,0) without touching h
            tmp = mlp_p.tile([V, D], bf16)
            nc.vector.scalar_tensor_tensor(
                out=tmp, in0=w1r0b, scalar=h_s[0:V, :], in1=b1bb,
                op0=alu.mult, op1=alu.add,
            )
            hpre = mlp_p.tile([V, D], bf16)
            nc.vector.scalar_tensor_tensor(
                out=hpre, in0=w1r1b, scalar=h_d[0:V, :], in1=tmp,
                op0=alu.mult, op1=alu.add,
            )
            hrelus[i] = hpre

        if 1 <= i <= R:
            # stage D (row i-1): table[v,:] = relu(...) @ W2
            r = i - 1
            hpre_r = hrelus.pop(r)
            pst = ps_mlp.tile([D, V], bf16, tag="mlp")
            nc.tensor.transpose(pst, hpre_r, identb)
            hT = mlp_p.tile([D, V], bf16)
            nc.scalar.activation(
                out=hT, in_=pst, func=mybir.ActivationFunctionType.Relu
            )
            pstab = ps_mlp.tile([V, D], f32, tag="mlp")
            nc.tensor.matmul(out=pstab, lhsT=hT, rhs=w2b)
            tab = mlp_p.tile([V, D], bf16)
            nc.scalar.copy(out=tab, in_=pstab)
            tabs[r] = tab

        if i >= 2:
            # stage E/F (row i-2): gather + psum->sbuf (+b2) + DMA per 2 rows
            r = i - 2
            oh = ohs.pop(r)
            tab = tabs.pop(r)
            ps_gs = ps_g.tile([128, S], f32, bufs=3)
            ps_gd = ps_g.tile([128, S], f32)
            nc.tensor.matmul(out=ps_gs[0:D, :], lhsT=tab, rhs=oh[0:V, 0:S])
            nc.tensor.matmul(out=ps_gd[0:D, :], lhsT=tab, rhs=oh[0:V, S : 2 * S])

            g = r % 2
            if g == 0:
                gout = gout_p.tile([D, 4 * S], f32)
            nc.scalar.activation(
                out=gout[:, g * S : (g + 1) * S], in_=ps_gs[0:D, :],
                func=mybir.ActivationFunctionType.Identity, bias=b2c[0:D, :],
                scale=1.0,
            )
            nc.scalar.activation(
                out=gout[:, (2 + g) * S : (3 + g) * S], in_=ps_gd[0:D, :],
                func=mybir.ActivationFunctionType.Identity, bias=b2c[0:D, :],
                scale=1.0,
            )
            if g == 1:
                win = slice((r - 1) * S, (r + 1) * S)
                nc.gpsimd.dma_start(out=out[0:D, win], in_=gout[:, 0 : 2 * S])
                nc.gpsimd.dma_start(
                    out=out[D : 2 * D, win], in_=gout[:, 2 * S : 4 * S]
                )


def _build_module():
    from contextlib import ExitStack

    from concourse import bacc, bass, tile

    mybir = bass.mybir
    # Bacc.finalize() runs generate_event_semaphores, splitting sync waits
    # to the HW limit of 1 per instruction (raw Bass skips that pass)
    nc = bacc.Bacc(
        "TRN2", target_bir_lowering=False, debug=False, num_devices=NCORES
    )
    idsf = nc.dram_tensor(
        "idsf", [R, 2 * S], mybir.dt.bfloat16, kind="ExternalInput"
    ).ap()
    consts = nc.dram_tensor(
        "consts", [128, CW], mybir.dt.float32, kind="ExternalInput"
    ).ap()
    out = nc.dram_tensor(
        "out", [128, R * S], mybir.dt.float32, kind="ExternalOutput"
    ).ap()

    with tile.TileContext(nc) as tc:
        with ExitStack() as ctx:
            _emit(ctx, nc, tc, idsf, consts, out)
    nc.finalize()
    return nc


def get_module():
    if "nc" not in _CACHE:
        _CACHE["nc"] = _build_module()
    return _CACHE["nc"]


def _build_consts(W1, b1, W2, b2):
    c = np.zeros((128, CW), np.float32)
    c[:, C_W1R0 : C_W1R0 + D] = W1[0]
    c[:, C_W1R1 : C_W1R1 + D] = W1[1]
    c[0, C_W1R0 : C_W1R0 + D] = 0.0
    c[0, C_W1R1 : C_W1R1 + D] = 0.0
    c[:, C_B1 : C_B1 + D] = b1
    c[0:D, C_W2 : C_W2 + D] = W2
    c[:, C_ID : C_ID + 128] = np.eye(128, dtype=np.float32)
    c[:, C_VIDX] = np.arange(128, dtype=np.float32)
    c[0:D, C_B2C] = b2
    c[D : 2 * D, C_B2C] = b2
    return c


def kernel(**inputs):
    global LAST_RESULT
    import ml_dtypes

    from concourse import bass_utils

    src = np.asarray(inputs["src_neighbor_ids"])
    dst = np.asarray(inputs["dst_neighbor_ids"])
    W1 = np.asarray(inputs["W1"], np.float32)
    b1 = np.asarray(inputs["b1"], np.float32)
    W2 = np.asarray(inputs["W2"], np.float32)
    b2 = np.asarray(inputs["b2"], np.float32)

    consts = _build_consts(W1, b1, W2, b2)
    idsf = np.concatenate(
        [src.astype(np.float32), dst.astype(np.float32)], axis=1
    ).astype(ml_dtypes.bfloat16)

    in_maps = []
    for c in range(NCORES):
        sl = slice(c * R, (c + 1) * R)
        in_maps.append({"idsf": idsf[sl], "consts": consts})

    nc = get_module()
    import os

    trace = bool(int(os.environ.get("KERNEL_TRACE", "0")))
    res = bass_utils.run_bass_kernel_spmd(
        nc, in_maps, core_ids=list(range(NCORES)), trace=trace
    )
    LAST_RESULT = res

    src_feat = np.empty((B, S, D), np.float32)
    dst_feat = np.empty((B, S, D), np.float32)
    for c in range(NCORES):
        o = res.results[c]["out"].reshape(128, R, S)
        sl = slice(c * R, (c + 1) * R)
        src_feat[sl] = o[0:D].transpose(1, 2, 0)
        dst_feat[sl] = o[D : 2 * D].transpose(1, 2, 0)
    return src_feat, dst_feat

